# revision 2
# baseline (speedup 1.0000x reference)
"""CRF NLL loss v1: lean meet-in-the-middle vector scan, no renorm.

Per core (64 seq): linear-space forward+backward chains packed in one
[64,64] tile; 1024 serial ticks of matmul + DVE multiply. Emission
exp/transposes pipelined off the critical path. Gold emission score via
one-hot DVE ops, folded with host-side transition score.
"""
import numpy as np

TAGSET = 32
START = 30
STOP = 31
B = 512
S = 2048
NCORES = 8
BC = B // NCORES          # 64 sequences per core
HALF = S // 2             # 1024 ticks per direction
CH = 64                   # emission steps per streamed chunk
NCH = HALF // CH          # 16 chunks per direction
MU = np.float32(4.3226)   # mean log-growth per step

_CACHE = {}


def _build_nc():
    import concourse.bacc as bacc
    import concourse.tile as tile
    from concourse import mybir

    f32 = mybir.dt.float32
    i32 = mybir.dt.int32
    AF = mybir.ActivationFunctionType
    OP = mybir.AluOpType
    AX = mybir.AxisListType

    nc = bacc.Bacc("TRN2", target_bir_lowering=False, debug=False,
                   num_devices=NCORES)

    em_d = nc.dram_tensor("emissions", [BC, S, TAGSET], f32,
                          kind="ExternalInput").ap()
    tr_d = nc.dram_tensor("transitions", [TAGSET, TAGSET], f32,
                          kind="ExternalInput").ap()
    nll_d = nc.dram_tensor("nll", [1, BC], f32, kind="ExternalOutput").ap()

    with tile.TileContext(nc) as tc:
        with (
            tc.tile_pool(name="const", bufs=1) as cp,
            tc.tile_pool(name="chunk", bufs=3) as ccp,
            tc.tile_pool(name="oh", bufs=2) as ohp,
            tc.tile_pool(name="xt", bufs=12) as xtp,
            tc.tile_pool(name="state", bufs=4) as stp,
            tc.tile_pool(name="small", bufs=2) as smp,
            tc.tile_pool(name="trp", bufs=3, space="PSUM") as trp,
            tc.tile_pool(name="mmp", bufs=2, space="PSUM") as mmp,
            tc.tile_pool(name="finp", bufs=1, space="PSUM") as fip,
        ):
            # ---------------- setup: weights, identity, ones ----------------
            w = cp.tile([64, 64], f32)
            nc.vector.memset(w[:], 0.0)
            # fwd block: w[p, t] = trans[t, p]  (strided transpose DMA, tiny)
            nc.sync.dma_start(w[0:32, 0:32], tr_d.rearrange("a b -> b a"))
            # bwd block: w[32+p, 32+t] = trans[p, t]
            nc.sync.dma_start(w[32:64, 32:64], tr_d)
            nc.vector.tensor_scalar_max(w[:], w[:], -80.0)
            nc.scalar.activation(w[:], w[:], AF.Exp)
            nc.vector.memset(w[0:32, 32:64], 0.0)
            nc.vector.memset(w[32:64, 0:32], 0.0)

            ones_t = cp.tile([64, 64], f32)
            nc.vector.memset(ones_t[:], 1.0)
            negmu = cp.tile([64, 1], f32)
            nc.vector.memset(negmu[:], -float(MU))
            ident = cp.tile([64, 64], f32)
            nc.gpsimd.affine_select(
                out=ident[:], in_=ones_t[:], pattern=[[-1, 64]],
                compare_op=OP.is_equal, fill=0.0, base=0, channel_multiplier=1)

            # ---------------- scan state init ----------------
            state = stp.tile([64, 64], f32, tag="state")
            nc.gpsimd.affine_select(
                out=state[0:32, :], in_=ones_t[0:32, :], pattern=[[0, 64]],
                compare_op=OP.is_equal, fill=0.0, base=-START,
                channel_multiplier=1)
            nc.gpsimd.affine_select(
                out=state[32:64, :], in_=ones_t[32:64, :], pattern=[[0, 64]],
                compare_op=OP.is_equal, fill=0.0, base=-STOP,
                channel_multiplier=1)

            # ---------------- main scan ----------------
            comb = None
            for tau in range(HALF):
                if tau % CH == 0:
                    g = tau // CH
                    comb = ccp.tile([BC, 2 * CH * TAGSET], f32, tag="comb")
                    cv = comb[:].rearrange("b (s u t) -> b s u t",
                                           u=2, t=TAGSET)
                    nc.sync.dma_start(cv[:, :, 0, :],
                                      em_d[:, g * CH:(g + 1) * CH, :])
                    nc.sync.dma_start(
                        cv[:, :, 1, :],
                        em_d[:, S - 1 - g * CH:S - (g + 1) * CH - 1:-1, :])
                l = tau % CH

                tr_ps = trp.tile([64, 64], f32, tag="trps")
                nc.tensor.transpose(tr_ps[:], comb[:, l * 64:(l + 1) * 64],
                                    ident[:])
                xt = xtp.tile([64, 64], f32, tag="xt")
                nc.scalar.activation(xt[:], tr_ps[:], AF.Exp, bias=negmu[:])

                ps = mmp.tile([64, 64], f32, tag="mm")
                nc.tensor.matmul(ps[:], w[:], state[:], start=True, stop=True)
                nstate = stp.tile([64, 64], f32, tag="state")
                nc.vector.tensor_mul(nstate[:], ps[:], xt[:])
                state = nstate

                if tau == HALF // 2:
                    # single mid-scan renorm: scale fwd/bwd halves by the
                    # reciprocal of a proxy row; log(scale) folded in later.
                    rec = smp.tile([64, 64], f32, tag="rec")
                    nc.vector.reciprocal(rec[0:1, :], state[0:1, :])
                    nc.vector.reciprocal(rec[32:33, :], state[32:33, :])
                    bc_ps = fip.tile([64, 64], f32, tag="bc")
                    nc.tensor.matmul(bc_ps[0:32, :], ones_t[0:1, 0:32],
                                     rec[0:1, :], start=True, stop=True)
                    nc.tensor.matmul(bc_ps[32:64, :], ones_t[32:33, 0:32],
                                     rec[32:33, :], start=True, stop=True,
                                     tile_position=(32, 32))
                    offacc = cp.tile([64, 64], f32)
                    nc.scalar.activation(offacc[0:1, :], state[0:1, :], AF.Ln)
                    nc.scalar.activation(offacc[32:33, :], state[32:33, :],
                                         AF.Ln)
                    rstate = stp.tile([64, 64], f32, tag="state")
                    nc.vector.tensor_mul(rstate[:], state[:], bc_ps[:])
                    state = rstate

            # ---------------- finale ----------------
            wb = cp.tile([64, 64], f32)
            nc.vector.memset(wb[:], 0.0)
            nc.sync.dma_start(wb[32:64, 0:32], w[32:64, 32:64])
            psf = mmp.tile([64, 64], f32, tag="mm")
            nc.tensor.matmul(psf[0:32, :], wb[32:64, 0:32], state[32:64, :],
                             start=True, stop=True)
            zp = smp.tile([64, 64], f32, tag="zp")
            nc.vector.tensor_mul(zp[0:32, :], psf[0:32, :], state[0:32, :])
            zsum = fip.tile([1, 64], f32, tag="zsum")
            nc.tensor.matmul(zsum[0:1, :], ones_t[0:32, 0:1], zp[0:32, :],
                             start=True, stop=True)
            lz = smp.tile([64, 64], f32, tag="lz")
            nc.scalar.activation(lz[0:1, :], zsum[0:1, :], AF.Ln)
            ob = smp.tile([64, 64], f32, tag="ob")
            nc.sync.dma_start(ob[0:1, :], offacc[32:33, :])
            nc.vector.tensor_add(lz[0:1, :], lz[0:1, :], offacc[0:1, :])
            nc.vector.tensor_add(lz[0:1, :], lz[0:1, :], ob[0:1, :])
            # logZ = lz + MU*S  (gold score subtracted on host)
            nc.vector.tensor_scalar_add(lz[0:1, :], lz[0:1, :],
                                        float(MU) * S)
            nc.sync.dma_start(nll_d, lz[0:1, :])

    nc.compile()
    return nc


def _get_nc():
    if "nc" not in _CACHE:
        _CACHE["nc"] = _build_nc()
    return _CACHE["nc"]


def kernel(emissions, transitions, tags):
    from concourse.bass_utils import run_bass_kernel_spmd

    em = np.ascontiguousarray(np.asarray(emissions, dtype=np.float32))
    tr = np.ascontiguousarray(np.asarray(transitions, dtype=np.float32))
    tg = np.ascontiguousarray(np.asarray(tags, dtype=np.int32))

    nc = _get_nc()
    in_maps = [
        {
            "emissions": em[c * BC:(c + 1) * BC],
            "transitions": tr,
        }
        for c in range(NCORES)
    ]
    res = run_bass_kernel_spmd(nc, in_maps, list(range(NCORES)))
    logz = np.concatenate([res.results[c]["nll"][0] for c in range(NCORES)])
    e_sc = np.take_along_axis(em, tg[:, :, None], axis=2)[..., 0].sum(axis=1)
    t_sc = (tr[tg[:, 1:], tg[:, :-1]].sum(axis=1)
            + tr[tg[:, 0], START] + tr[STOP, tg[:, -1]])
    total = (np.sum(logz.astype(np.float64)) - np.sum(e_sc.astype(np.float64))
             - np.sum(t_sc.astype(np.float64)))
    return np.array(total, dtype=np.float32)
